# revision 1
# baseline (speedup 1.0000x reference)
"""ResNet BasicBlock (conv3x3-BN-ReLU-conv3x3-BN-add-ReLU) on 8 Trainium2 cores.

Strategy:
  - Pure data parallel: batch 32 -> 4 images per core; weights/BN replicated.
  - BN folded into conv weights on host (w *= gamma*rsqrt(var+eps); bias terms
    kept separate, applied on-chip per output-channel partition).
  - Conv3x3 = 9 shifted 1x1 convs = matmuls accumulated in PSUM:
      out[O, spatial] += wT[I(128part), O] @ x_shift[I(128part), spatial]
    with channels on partitions (256 ch = 2 blocks of 128), spatial chunked
    into 8 rows x 56 cols = 448 columns per PSUM bank.
  - Inputs padded to 58x58 on host (zeros), conv1 output padded on-chip, so
    shifted windows are plain strided APs.
  - fp16 matmul operands (same PE rate as bf16, ~8x better precision),
    fp32 PSUM accumulation, fp32 epilogues and output.
"""

import numpy as np

import concourse.mybir as mybir
import concourse.tile as tile
from concourse import bacc
from concourse.bass_utils import run_bass_kernel_spmd

EPS = 1e-5
NCORES = 8
N, C, H, W = 32, 256, 56, 56
NPC = N // NCORES          # images per core
HP, WP = H + 2, W + 2      # padded spatial
CB = C // 128              # channel blocks (2)
RC = 8                     # rows per PSUM chunk
NCHUNK = H // RC           # 7 chunks
F16 = mybir.dt.float16
F32 = mybir.dt.float32

_CACHE = {}


def _build():
    nc = bacc.Bacc("TRN2", target_bir_lowering=False, debug=False,
                   num_devices=NCORES)
    xp = nc.dram_tensor("xp", [NPC, CB, 128, HP, WP], F16,
                        kind="ExternalInput").ap()
    w1t = nc.dram_tensor("w1t", [CB, 128, 9, C], F16, kind="ExternalInput").ap()
    w2t = nc.dram_tensor("w2t", [CB, 128, 9, C], F16, kind="ExternalInput").ap()
    b1 = nc.dram_tensor("b1", [CB, 128, 1], F32, kind="ExternalInput").ap()
    b2 = nc.dram_tensor("b2", [CB, 128, 1], F32, kind="ExternalInput").ap()
    y = nc.dram_tensor("y", [NPC, CB, 128, H, W], F32,
                       kind="ExternalOutput").ap()

    Relu = mybir.ActivationFunctionType.Relu
    Add = mybir.AluOpType.add

    with tile.TileContext(nc) as tc:
        with tc.tile_pool(name="w", bufs=1) as wp, \
             tc.tile_pool(name="x", bufs=3) as xpool, \
             tc.tile_pool(name="h", bufs=1) as hpool, \
             tc.tile_pool(name="yst", bufs=2) as ypool, \
             tc.tile_pool(name="tmp", bufs=4) as tpool, \
             tc.tile_pool(name="ps", bufs=8, space="PSUM") as pspool:

            # Startup: DMA issues serialize at ~620ns each on the Sync queue,
            # so order by first-need. The first matmul group (ob=0, chunk 0)
            # needs only x0 rows 0:10 and the ob=0 half of w1; Tile tracks
            # subtile ranges, so finer pieces unblock the PE sooner.
            # conv2's weights aren't needed for ~100us -> DMA'd after conv1(0).
            hh = HP // 2
            w1s, w2s, b1s, b2s = [], [], [], []
            xt0 = [xpool.tile([128, HP, WP], F16, tag=f"x{ib}", name=f"xt0_{ib}")
                   for ib in range(CB)]
            for ib in range(CB):
                t = wp.tile([128, 9, C], F16, tag=f"w1_{ib}")
                w1s.append(t)
            for ib in range(CB):
                nc.sync.dma_start(out=xt0[ib][:, :10, :],
                                  in_=xp[0, ib, :, :10, :])
                nc.sync.dma_start(out=w1s[ib][:, :, :128],
                                  in_=w1t[ib, :, :, :128])
            for ib in range(CB):
                nc.sync.dma_start(out=xt0[ib][:, 10:hh, :],
                                  in_=xp[0, ib, :, 10:hh, :])
            for ib in range(CB):
                nc.sync.dma_start(out=xt0[ib][:, hh:, :],
                                  in_=xp[0, ib, :, hh:, :])
                nc.sync.dma_start(out=w1s[ib][:, :, 128:],
                                  in_=w1t[ib, :, :, 128:])
                t = wp.tile([128, 1], F32, tag=f"b1_{ib}")
                nc.sync.dma_start(out=t[:], in_=b1[ib])
                b1s.append(t)

            def load_w2():
                for ib in range(CB):
                    t = wp.tile([128, 9, C], F16, tag=f"w2_{ib}")
                    nc.sync.dma_start(out=t[:], in_=w2t[ib])
                    w2s.append(t)
                    t = wp.tile([128, 1], F32, tag=f"b2_{ib}")
                    nc.sync.dma_start(out=t[:], in_=b2[ib])
                    b2s.append(t)

            # PE warmup: the HAM clock gate holds the PE at 1.2 GHz until it
            # has been busy ~3.4us. The PE is idle during the initial DMA
            # wait anyway, so run throwaway matmuls on a zeroed scratch tile
            # to unthrottle the clock before the first real matmul.
            scratch = wp.tile([128, RC * W], F16, tag="warm_scratch")
            nc.gpsimd.memset(scratch[:], 0.0)
            # 16 cold matmuls: trips the ~3.4us HAM window AND keeps the PE
            # occupied until the second channel block's x/w DMAs have landed
            # (8 warmups measured 1.7us slower: the PE arrived early and
            # stalled on the ib=1 input DMA instead)
            ps_w = pspool.tile([128, RC * W], F32, name="ps_warm", tag="ps")
            for _ in range(16):
                nc.tensor.matmul(ps_w[:], scratch[:, :128], scratch[:],
                                 start=True, stop=True)

            # persistent conv1-output tiles: 2 channel blocks x 2 pipeline
            # parities. Fully zeroed ONCE here (so the 1-px border is zero
            # and no uninitialized element is ever read); the interior is
            # overwritten by conv1's epilogue every image.
            hts_all = {}
            for par in range(2):
                for ob in range(CB):
                    t = hpool.tile([128, HP, WP], F16, tag=f"h{par}_{ob}")
                    nc.vector.memset(t[:], 0.0)
                    hts_all[(par, ob)] = t

            def load_x(img):
                xt = []
                for ib in range(CB):
                    t = xpool.tile([128, HP, WP], F16, tag=f"x{ib}")
                    nc.sync.dma_start(out=t[:, :hh, :], in_=xp[img, ib, :, :hh, :])
                    nc.sync.dma_start(out=t[:, hh:, :], in_=xp[img, ib, :, hh:, :])
                    xt.append(t)
                return xt

            def conv1(img, xt):
                ht = [hts_all[(img % 2, ob)] for ob in range(CB)]
                for ob in range(CB):
                    for c in range(NCHUNK):
                        r0 = RC * c
                        ps = pspool.tile([128, RC, W], F32)
                        k = 0
                        for ib in range(CB):
                            for kx in range(3):
                                for ky in range(3):
                                    nc.tensor.matmul(
                                        ps[:],
                                        w1s[ib][:, 3 * ky + kx,
                                                128 * ob:128 * ob + 128],
                                        xt[ib][:, r0 + ky:r0 + ky + RC,
                                               kx:kx + W],
                                        start=(k == 0), stop=(k == 17))
                                    k += 1
                        nc.scalar.activation(
                            ht[ob][:, 1 + r0:1 + r0 + RC, 1:1 + W], ps[:],
                            Relu, bias=b1s[ob][:], scale=1.0)
                return ht

            def conv2(img, xt, ht):
                for ob in range(CB):
                    yt = ypool.tile([128, H, W], F32, tag=f"y{ob}")
                    # the very last group of the kernel sits on the critical
                    # path (MMs -> add -> relu -> DMA fully serial); split it
                    # into two half-height groups so the first half's
                    # epilogue overlaps the second half's matmuls
                    split_last = (img == NPC - 1 and ob == CB - 1)
                    groups = [(RC * c, RC) for c in range(NCHUNK - 1)]
                    if split_last:
                        groups += [(RC * (NCHUNK - 1), RC // 2),
                                   (RC * (NCHUNK - 1) + RC // 2, RC // 2)]
                    else:
                        groups += [(RC * (NCHUNK - 1), RC)]
                    pend = 0
                    for gi, (r0, nr) in enumerate(groups):
                        ps = pspool.tile([128, nr, W], F32, name="ps2",
                                         tag="ps")
                        k = 0
                        for ib in range(CB):
                            for kx in range(3):
                                for ky in range(3):
                                    nc.tensor.matmul(
                                        ps[:],
                                        w2s[ib][:, 3 * ky + kx,
                                                128 * ob:128 * ob + 128],
                                        ht[ib][:, r0 + ky:r0 + ky + nr,
                                               kx:kx + W],
                                        start=(k == 0), stop=(k == 17))
                                    k += 1
                        # residual add (identity = padded-x interior, fp16)
                        tmp = tpool.tile([128, nr, W], F32, name="tmp")
                        nc.vector.tensor_tensor(
                            out=tmp[:], in0=ps[:],
                            in1=xt[ob][:, 1 + r0:1 + r0 + nr, 1:1 + W],
                            op=Add)
                        # + per-channel bias, relu
                        nc.scalar.activation(
                            yt[:, r0:r0 + nr, :], tmp[:],
                            Relu, bias=b2s[ob][:], scale=1.0)
                        # stream the output out in row-group chunks so the
                        # final DMA isn't serialized after the last chunk
                        done = r0 + nr
                        if (done - pend >= 2 * RC or gi == len(groups) - 1
                                or (split_last and r0 >= RC * (NCHUNK - 1))):
                            nc.sync.dma_start(out=y[img, ob, :, pend:done, :],
                                              in_=yt[:, pend:done, :])
                            pend = done

            # software pipeline: conv1(i+1) emitted before conv2(i) so the PE
            # has independent work while conv2 waits on conv1's epilogue
            xts, hts = {}, {}
            xts[0] = xt0
            hts[0] = conv1(0, xts[0])
            load_w2()
            for img in range(1, NPC):
                xts[img] = load_x(img)
                hts[img] = conv1(img, xts[img])
                conv2(img - 1, xts[img - 1], hts[img - 1])
            conv2(NPC - 1, xts[NPC - 1], hts[NPC - 1])

    nc.compile()
    return nc


def _prep(inputs):
    x = np.asarray(inputs["x"], np.float32)
    out = {}
    for i in (1, 2):
        s = np.asarray(inputs[f"g{i}"], np.float32) / np.sqrt(
            np.asarray(inputs[f"rv{i}"], np.float32) + EPS)
        b = (np.asarray(inputs[f"b{i}"], np.float32)
             - np.asarray(inputs[f"rm{i}"], np.float32) * s)
        w = np.asarray(inputs[f"w{i}"], np.float32) * s[:, None, None, None]
        # [O,I,3,3] -> [I, ky, kx, O] -> [CB, 128, 9, O]
        wt = np.ascontiguousarray(w.transpose(1, 2, 3, 0)).reshape(
            C, 9, C).reshape(CB, 128, 9, C).astype(np.float16)
        out[f"w{i}t"] = wt
        out[f"b{i}"] = np.ascontiguousarray(b.reshape(CB, 128, 1))
    xpad = np.zeros((N, C, HP, WP), np.float16)
    xpad[:, :, 1:-1, 1:-1] = x
    out["xp"] = xpad.reshape(NCORES, NPC, CB, 128, HP, WP)
    return out


def run(inputs, trace=False):
    if "nc" not in _CACHE:
        _CACHE["nc"] = _build()
    nc = _CACHE["nc"]
    p = _prep(inputs)
    in_maps = [{"xp": p["xp"][c], "w1t": p["w1t"], "w2t": p["w2t"],
                "b1": p["b1"], "b2": p["b2"]} for c in range(NCORES)]
    res = run_bass_kernel_spmd(nc, in_maps, core_ids=list(range(NCORES)),
                               trace=trace)
    yout = np.concatenate(
        [r["y"].reshape(NPC, C, H, W) for r in res.results], axis=0)
    return yout, res


def kernel(**inputs):
    yout, _ = run(inputs)
    return yout



# revision 3
# speedup vs baseline: 1.3630x; 1.3630x over previous
"""ResNet BasicBlock (conv3x3-BN-ReLU-conv3x3-BN-add-ReLU) on 8 Trainium2 cores.

Strategy:
  - Pure data parallel: batch 32 -> 4 images per core; weights/BN replicated.
  - BN folded into conv weights on host; bias applied in the epilogue.
  - 1D Winograd F(2,3) along W: 1.5x fewer PE MACs than direct conv.
      V_b = (B^T d)_b  computed on DVE as stride-2 slice add/subs of the
            padded input (4 ops per channel block, fp16 2x mode)
      m_b = sum_{ky,ib} wtil_b[ky]^T @ V_b   (PE, PSUM fp32, 6 matmuls per
            m; 24 matmuls x 392 cols per 14-row chunk vs direct 18 x 784)
      y_even = m0+m1+m2, y_odd = m1-m2-m3   (Scalar copies m1,m2 to SBUF
            fp16; DVE combines, reading m0/m3 straight from PSUM)
    Weight transform (G along kx) and BN fold are done on host.
  - fp16 matmul operands, fp32 PSUM accumulation; epilogue combines in
    fp16 (err ~1e-3 vs fp32 reference, gate is 2e-2).
  - Inputs padded to 58x58 on host (zeros); conv1 output (h) kept padded
    on-chip so conv2's V transform is identical to conv1's.
"""

import numpy as np

import concourse.mybir as mybir
import concourse.tile as tile
from concourse import bacc
from concourse.bass_utils import run_bass_kernel_spmd

EPS = 1e-5
NCORES = 8
N, C, H, W = 32, 256, 56, 56
NPC = N // NCORES          # images per core
HP, WP = H + 2, W + 2      # padded spatial
CB = C // 128              # channel blocks (2)
RC = 14                    # rows per PSUM chunk
NCHUNK = H // RC           # 4 chunks
T = W // 2                 # winograd tiles per row (28)
F16 = mybir.dt.float16
F32 = mybir.dt.float32

_CACHE = {}


def _build():
    nc = bacc.Bacc("TRN2", target_bir_lowering=False, debug=False,
                   num_devices=NCORES)
    xp = nc.dram_tensor("xp", [NPC, CB, 128, HP, WP], F16,
                        kind="ExternalInput").ap()
    w1t = nc.dram_tensor("w1t", [CB, 128, 12, C], F16, kind="ExternalInput").ap()
    w2t = nc.dram_tensor("w2t", [CB, 128, 12, C], F16, kind="ExternalInput").ap()
    b1 = nc.dram_tensor("b1", [CB, 128, 1], F32, kind="ExternalInput").ap()
    b2 = nc.dram_tensor("b2", [CB, 128, 1], F32, kind="ExternalInput").ap()
    y = nc.dram_tensor("y", [NPC, CB, 128, H, W], F32,
                       kind="ExternalOutput").ap()

    Relu = mybir.ActivationFunctionType.Relu
    Copy = mybir.ActivationFunctionType.Copy
    Add = mybir.AluOpType.add
    Sub = mybir.AluOpType.subtract

    with tile.TileContext(nc) as tc:
        with tc.tile_pool(name="w", bufs=1) as wp, \
             tc.tile_pool(name="x", bufs=3) as xpool, \
             tc.tile_pool(name="v", bufs=1) as vpool, \
             tc.tile_pool(name="h", bufs=1) as hpool, \
             tc.tile_pool(name="yst", bufs=4) as ypool, \
             tc.tile_pool(name="tmp", bufs=4) as tpool, \
             tc.tile_pool(name="ps", bufs=8, space="PSUM") as pspool:

            # Startup: DMA issues serialize at ~620ns each on the Sync queue,
            # so order by first-need: x (feeds the V transform, the longest
            # dependency chain) before weights.
            hh = HP // 2
            w1s, w2s, b1s, b2s = [], [], [], []
            xt0 = [xpool.tile([128, HP, WP], F16, tag=f"x{ib}", name=f"xt0_{ib}")
                   for ib in range(CB)]
            for ib in range(CB):
                nc.sync.dma_start(out=xt0[ib][:, :hh, :],
                                  in_=xp[0, ib, :, :hh, :])
                nc.sync.dma_start(out=xt0[ib][:, hh:, :],
                                  in_=xp[0, ib, :, hh:, :])
            for ib in range(CB):
                t = wp.tile([128, 12, C], F16, tag=f"w1_{ib}")
                w1s.append(t)
                nc.sync.dma_start(out=w1s[ib][:, :, :128],
                                  in_=w1t[ib, :, :, :128])
            for ib in range(CB):
                nc.sync.dma_start(out=w1s[ib][:, :, 128:],
                                  in_=w1t[ib, :, :, 128:])
                t = wp.tile([128, 1], F32, tag=f"b1_{ib}")
                nc.sync.dma_start(out=t[:], in_=b1[ib])
                b1s.append(t)

            def load_w2():
                for ib in range(CB):
                    t = wp.tile([128, 12, C], F16, tag=f"w2_{ib}")
                    nc.sync.dma_start(out=t[:], in_=w2t[ib])
                    w2s.append(t)
                    t = wp.tile([128, 1], F32, tag=f"b2_{ib}")
                    nc.sync.dma_start(out=t[:], in_=b2[ib])
                    b2s.append(t)

            # PE warmup: the HAM clock gate holds the PE at 1.2 GHz until it
            # has been busy ~3.4us. The PE is idle during the initial DMA +
            # V-transform wait anyway, so run throwaway matmuls on a zeroed
            # scratch tile to unthrottle the clock before the first real one.
            scratch = wp.tile([128, RC * T], F16, tag="warm_scratch")
            nc.gpsimd.memset(scratch[:], 0.0)
            ps_w = pspool.tile([128, RC * T], F32, name="ps_warm", tag="ps")
            for _ in range(16):
                nc.tensor.matmul(ps_w[:], scratch[:, :128], scratch[:],
                                 start=True, stop=True)

            # persistent conv1-output tiles: 2 channel blocks x 2 pipeline
            # parities, fully zeroed ONCE (the 1-px border must stay zero);
            # the interior is overwritten by conv1's epilogue every image.
            # memset on gpsimd to keep the DVE free for the V transforms.
            hts_all = {}
            for par in range(2):
                for ob in range(CB):
                    t = hpool.tile([128, HP, WP], F16, tag=f"h{par}_{ob}")
                    nc.gpsimd.memset(t[:], 0.0)
                    hts_all[(par, ob)] = t

            # persistent Winograd-transform tiles (single-buffered: the
            # transform for image i+1 WAR-waits on the conv reads of image i,
            # and the DVE has plenty of slack to hide that)
            v1s = [vpool.tile([128, 4, HP, T], F16, tag=f"v1_{ib}",
                              name=f"v1_{ib}") for ib in range(CB)]
            v2s = [vpool.tile([128, 4, HP, T], F16, tag=f"v2_{ib}",
                              name=f"v2_{ib}") for ib in range(CB)]

            def vtransform(vt, src):
                # V_b = (B^T d)_b over W for every padded row, as stride-2
                # whole-tile slice ops (fp16, all-SBUF -> DVE 2x mode)
                for ib in range(CB):
                    s, v = src[ib], vt[ib]
                    nc.vector.tensor_tensor(out=v[:, 0], in0=s[:, :, 0:56:2],
                                            in1=s[:, :, 2:58:2], op=Sub)
                    nc.vector.tensor_tensor(out=v[:, 1], in0=s[:, :, 1:57:2],
                                            in1=s[:, :, 2:58:2], op=Add)
                    nc.vector.tensor_tensor(out=v[:, 2], in0=s[:, :, 2:58:2],
                                            in1=s[:, :, 1:57:2], op=Sub)
                    nc.vector.tensor_tensor(out=v[:, 3], in0=s[:, :, 1:57:2],
                                            in1=s[:, :, 3:58:2], op=Sub)

            def load_x(img):
                xt = []
                for ib in range(CB):
                    t = xpool.tile([128, HP, WP], F16, tag=f"x{ib}")
                    nc.sync.dma_start(out=t[:, :hh, :], in_=xp[img, ib, :, :hh, :])
                    nc.sync.dma_start(out=t[:, hh:, :], in_=xp[img, ib, :, hh:, :])
                    xt.append(t)
                return xt

            def wino_groups(vt, ws, ob, r0, nr):
                ps = []
                for b in range(4):
                    p = pspool.tile([128, nr, T], F32, name="ps", tag="ps")
                    k = 0
                    for ib in range(CB):
                        for ky in range(3):
                            nc.tensor.matmul(
                                p[:],
                                ws[ib][:, 3 * b + ky, 128 * ob:128 * ob + 128],
                                vt[ib][:, b, r0 + ky:r0 + ky + nr, :],
                                start=(k == 0), stop=(k == 5))
                            k += 1
                    ps.append(p)
                return ps

            def combine(ps, nr):
                # y_even = m0+m1+m2, y_odd = m1-m2-m3. Scalar copies the
                # shared m1/m2 to SBUF fp16; DVE reads m0/m3 from PSUM.
                t1 = tpool.tile([128, nr, T], F16, name="t1")
                t2 = tpool.tile([128, nr, T], F16, name="t2")
                nc.scalar.activation(t1[:], ps[1][:], Copy)
                nc.scalar.activation(t2[:], ps[2][:], Copy)
                e1 = tpool.tile([128, nr, T], F16, name="e1")
                nc.vector.tensor_tensor(out=e1[:], in0=ps[0][:], in1=t1[:],
                                        op=Add)
                e = tpool.tile([128, nr, T], F16, name="e")
                nc.vector.tensor_tensor(out=e[:], in0=e1[:], in1=t2[:], op=Add)
                o1 = tpool.tile([128, nr, T], F16, name="o1")
                nc.vector.tensor_tensor(out=o1[:], in0=t1[:], in1=t2[:], op=Sub)
                o = tpool.tile([128, nr, T], F16, name="o")
                nc.vector.tensor_tensor(out=o[:], in0=o1[:], in1=ps[3][:],
                                        op=Sub)
                return e, o

            def conv1(img, xt):
                ht = [hts_all[(img % 2, ob)] for ob in range(CB)]
                for ob in range(CB):
                    for c in range(NCHUNK):
                        r0 = RC * c
                        ps = wino_groups(v1s, w1s, ob, r0, RC)
                        e, o = combine(ps, RC)
                        nc.scalar.activation(
                            ht[ob][:, 1 + r0:1 + r0 + RC, 1:56:2], e[:],
                            Relu, bias=b1s[ob][:], scale=1.0)
                        nc.scalar.activation(
                            ht[ob][:, 1 + r0:1 + r0 + RC, 2:57:2], o[:],
                            Relu, bias=b1s[ob][:], scale=1.0)
                return ht

            def conv2(img, xt):
                for ob in range(CB):
                    for c in range(NCHUNK):
                        r0 = RC * c
                        ps = wino_groups(v2s, w2s, ob, r0, RC)
                        e, o = combine(ps, RC)
                        # residual add (identity = padded-x interior, fp16)
                        e2 = tpool.tile([128, RC, T], F16, name="e2")
                        nc.vector.tensor_tensor(
                            out=e2[:], in0=e[:],
                            in1=xt[ob][:, 1 + r0:1 + r0 + RC, 1:56:2], op=Add)
                        o2 = tpool.tile([128, RC, T], F16, name="o2")
                        nc.vector.tensor_tensor(
                            out=o2[:], in0=o[:],
                            in1=xt[ob][:, 1 + r0:1 + r0 + RC, 2:57:2], op=Add)
                        yt = ypool.tile([128, RC, W], F32, tag=f"y{ob}")
                        nc.scalar.activation(yt[:, :, 0:56:2], e2[:],
                                             Relu, bias=b2s[ob][:], scale=1.0)
                        nc.scalar.activation(yt[:, :, 1:56:2], o2[:],
                                             Relu, bias=b2s[ob][:], scale=1.0)
                        nc.sync.dma_start(out=y[img, ob, :, r0:r0 + RC, :],
                                          in_=yt[:])

            # software pipeline: conv1(i+1) emitted before conv2(i) so the PE
            # has independent work while conv2(i) waits on its V2 transform
            xts = {0: xt0}
            vtransform(v1s, xt0)
            conv1(0, xt0)
            load_w2()
            for img in range(1, NPC):
                xts[img] = load_x(img)
                vtransform(v1s, xts[img])
                conv1(img, xts[img])
                vtransform(v2s, [hts_all[((img - 1) % 2, ob)]
                                 for ob in range(CB)])
                conv2(img - 1, xts[img - 1])
            vtransform(v2s, [hts_all[((NPC - 1) % 2, ob)] for ob in range(CB)])
            conv2(NPC - 1, xts[NPC - 1])

    nc.compile()
    return nc


def _prep(inputs):
    x = np.asarray(inputs["x"], np.float32)
    G = np.array([[1, 0, 0], [.5, .5, .5], [.5, -.5, .5], [0, 0, 1]],
                 np.float32)
    out = {}
    for i in (1, 2):
        s = np.asarray(inputs[f"g{i}"], np.float32) / np.sqrt(
            np.asarray(inputs[f"rv{i}"], np.float32) + EPS)
        b = (np.asarray(inputs[f"b{i}"], np.float32)
             - np.asarray(inputs[f"rm{i}"], np.float32) * s)
        w = np.asarray(inputs[f"w{i}"], np.float32) * s[:, None, None, None]
        # winograd weight transform along kx: wwin[o,i,ky,b] = G[b,:] . w[o,i,ky,:]
        wwin = np.einsum('bk,oiyk->oiyb', G, w)
        # layout [CB, 128, 12, O] with tap index 3*b+ky
        wt = np.ascontiguousarray(wwin.transpose(1, 3, 2, 0)).reshape(
            C, 12, C).reshape(CB, 128, 12, C).astype(np.float16)
        out[f"w{i}t"] = wt
        out[f"b{i}"] = np.ascontiguousarray(b.reshape(CB, 128, 1))
    xpad = np.zeros((N, C, HP, WP), np.float16)
    xpad[:, :, 1:-1, 1:-1] = x
    out["xp"] = xpad.reshape(NCORES, NPC, CB, 128, HP, WP)
    return out


def run(inputs, trace=False):
    if "nc" not in _CACHE:
        _CACHE["nc"] = _build()
    nc = _CACHE["nc"]
    p = _prep(inputs)
    in_maps = [{"xp": p["xp"][c], "w1t": p["w1t"], "w2t": p["w2t"],
                "b1": p["b1"], "b2": p["b2"]} for c in range(NCORES)]
    res = run_bass_kernel_spmd(nc, in_maps, core_ids=list(range(NCORES)),
                               trace=trace)
    yout = np.concatenate(
        [r["y"].reshape(NPC, C, H, W) for r in res.results], axis=0)
    return yout, res


def kernel(**inputs):
    yout, _ = run(inputs)
    return yout
